# revision 37
# baseline (speedup 1.0000x reference)
"""MCR loss kernel for Trainium2 (8 NeuronCores).

Strategy:
  - Shard batch T=16 -> 2 timesteps per core (data parallel, no collectives).
  - Per core, on device: 8x8 avg-pool (as sum; the 1/64 is folded into the
    conv weights) via vector-engine reduces over uniform [128, 4608] slabs
    (24 input rows of 4 planes for the A-pass; 2 planes x 2 row-halves for
    the B-pass so all 128 partitions stay busy); the stage-2 row reduce
    writes straight into the reflect-padded conv input tile; reflect pad +
    3x3 conv as 3 PE matmuls with K=(dy,ic)=96; LeakyReLU(0.2) on the
    scalar engine from PSUM; Gram G_t = V_t V_t^T via PE transpose +
    matmul, contraction over the 576 pixels.
  - Host: matrix determinant lemma
        logdet(I_576 + a V^T V) = logdet(I_96 + a V V^T)
    so only the [2,96,96] Grams leave the device; float64 Cholesky logdets
    (16 x 96x96 + 48 x 32x32, ~5 MFLOP total) finish the scalar loss.
"""

import numpy as np

_STATE = {}

# -------- fixed problem geometry (hardcoded per harness contract) --------
B, CCH, H, W = 16, 32, 192, 192
NCORES = 8
TPC = B // NCORES          # timesteps per core = 2
OUT = 24                   # pooled spatial size
PIX = OUT * OUT            # 576
M = 96                     # feature rows (3 maps x 32 channels)
ALPHA_E = 6.0              # 576 / (96 * eps)
ALPHA_C = 18.0             # 576 / (32 * eps)

# plane order: A-pass planes stream first (all of t=1 plus t0m0) so the
# t=1 gram overlaps the B-pass stream; tail only holds 2 convs + gram t0.
PLANES_A = [(1, 0), (1, 1), (1, 2), (0, 0)]
PLANES_B = [(0, 1), (0, 2)]
PLANES = PLANES_A + PLANES_B

# If True, conv epilogue is a single scalar-engine Lrelu from PSUM.
# Fallback (False): scalar copy to SBUF + vector scalar_tensor_tensor max.
SCALAR_LRELU = False


def _build_nc():
    import concourse.bass as bass
    import concourse.tile as tile
    from concourse import bacc, mybir

    DT = mybir.dt.float32
    nc = bacc.Bacc(
        "TRN2", target_bir_lowering=False, debug=False, num_devices=NCORES
    )

    # xa: A planes; xb: B planes host-pre-shuffled to [s, h2, g, c, 24h, w]
    # so each B slab is one 128-partition DMA; wt[m,dx,(dy,ic),oc]
    x = nc.declare_dram_parameter("x", [4, CCH, H, W], DT, isOutput=False)
    xb = nc.declare_dram_parameter("xb", [4, 2, 2, CCH, 24, W], DT, isOutput=False)
    wt = nc.declare_dram_parameter("wt", [3, 3, 96, 32], DT, isOutput=False)
    ident = nc.declare_dram_parameter("ident", [128, 128], DT, isOutput=False)
    g_out = nc.declare_dram_parameter("g_out", [TPC, M, M], DT, isOutput=True)

    ACT = mybir.ActivationFunctionType
    XP = 26 * 26  # padded plane free size

    with tile.TileContext(nc) as tc:
        with (
            tc.tile_pool(name="persist", bufs=1) as persist,
            tc.tile_pool(name="slab", bufs=3) as slab_pool,
            tc.tile_pool(name="wsum", bufs=2) as wsum_pool,
            tc.tile_pool(name="xrep", bufs=4) as xrep_pool,
            tc.tile_pool(name="zc", bufs=12) as zc_pool,
            tc.tile_pool(name="vt", bufs=3) as vt_pool,
            tc.tile_pool(name="psum", bufs=2, space="PSUM") as psum_pool,
            tc.tile_pool(name="psumg", bufs=2, space="PSUM") as psumg_pool,
        ):
            wt_sb = persist.tile([96, 288], DT, tag="wt")
            nc.gpsimd.dma_start(
                out=wt_sb[:].rearrange("p (m x c) -> p m x c", m=3, x=3),
                in_=wt.ap().rearrange("m x p c -> p m x c"),
            )
            id_sb = persist.tile([128, 128], DT, tag="ident")
            nc.gpsimd.dma_start(out=id_sb[:], in_=ident.ap())

            # padded conv inputs: planes 0..3 in xpadA bands; planes 4,5 in
            # xpadB[0:64]; B's odd row-halves land at xpadB[64:128] and are
            # merged down by one gpsimd copy.
            xpadA = persist.tile([128, XP], DT, tag="xpadA")
            xpadB = persist.tile([128, XP], DT, tag="xpadB")
            v_sb = persist.tile([96, TPC * PIX], DT, tag="v")
            g_sb = persist.tile([96, TPC * 96], DT, tag="g")

            def stage1(slab, wsum, ny=3):
                # sum w in groups of 8 (contiguous innermost), writing wsum
                # in (y, x, r) layout so stage2's inner reads are contiguous
                nc.vector.tensor_reduce(
                    out=wsum.rearrange("p (y x r) -> p y r x", y=ny, x=24, r=8),
                    in_=slab.rearrange("p (g w) -> p g w", w=8),
                    axis=mybir.AxisListType.X,
                    op=mybir.AluOpType.add,
                )

            def stage2(wsum, xpad_dst, prange, ybase):
                # sum rows r in groups of 8 (contiguous innermost in the
                # permuted wsum), writing the 3 output rows straight into
                # the padded tile interior
                lo, n = prange
                nc.vector.tensor_reduce(
                    out=xpad_dst[lo : lo + n, :]
                    .rearrange("p (y x) -> p y x", y=26)[:, ybase : ybase + 3, 1:25],
                    in_=wsum[lo : lo + n, :].rearrange(
                        "p (y x r) -> p y x r", y=3, x=24, r=8
                    ),
                    axis=mybir.AxisListType.X,
                    op=mybir.AluOpType.add,
                )

            # ---- A pass: 8 slabs [128=(4 planes, 32c), 24h x 192w] ----
            # slab 0 is split into 3 row-chunks (DMA + stage1 each) so the
            # pipeline ramps as soon as the first 0.8 MB lands
            for s in range(8):
                slab = slab_pool.tile([128, 4608], DT, tag="slab")
                wsum = wsum_pool.tile([128, 576], DT, tag="wsum")
                nchunk = 3 if s == 0 else 1
                for k in range(nchunk):
                    r0, r1 = 24 * k // nchunk, 24 * (k + 1) // nchunk
                    nc.sync.dma_start(
                        out=slab[:, 192 * r0 : 192 * r1],
                        in_=x.ap()[0:4, :, 24 * s + r0 : 24 * s + r1, :]
                        .rearrange("g c h w -> (g c) h w"),
                    )
                    stage1(
                        slab[:, 192 * r0 : 192 * r1],
                        wsum[:, 24 * r0 : 24 * r1],
                        ny=(r1 - r0) // 8,
                    )
                stage2(wsum, xpadA, (0, 128), 1 + 3 * s)

            # reflect-pad edges for A planes (same-partition DVE copies)
            def edges(xpad, lo, n):
                x3 = xpad[lo : lo + n, :].rearrange("p (y x) -> p y x", y=26)
                # rows 0 and 25 <- interior rows 1 and 22 (xpad rows 2, 23)
                nc.vector.tensor_copy(
                    x3[:, 0:26:25, 1:25], x3[:, 2:24:21, 1:25]
                )
                # cols 0 and 25 <- cols 2 and 23 (covers corners too)
                nc.vector.tensor_copy(
                    x3[:, :, 0:26:25], x3[:, :, 2:24:21]
                )

            edges(xpadA, 0, 128)

            # ---- conv pieces ------------------------------------------------
            # conv: 3-row replicate on DVE (partition-shifted copies), 3 PE
            # matmuls per half with K=(dy,ic)=96, scalar copy PSUM->SBUF
            # (frees PSUM immediately), then LeakyReLU on gpsimd (SBUF-only
            # max(0.2*zc, zc)) into the v_sb band. Engine queues stay
            # decoupled from the B-slab pooling stream on the DVE.
            def conv_mm(t, m, xpad, lo):
                xrep = xrep_pool.tile([96, 24 * 26], DT, tag="xrep")
                for dy in range(3):
                    nc.vector.tensor_copy(
                        xrep[dy * 32 : (dy + 1) * 32, :],
                        xpad[lo : lo + 32, dy * 26 : dy * 26 + 624],
                    )
                xr3 = xrep[:].rearrange("p (y x) -> p y x", y=OUT, x=26)
                zcs = []
                for half in range(2):
                    pc = psum_pool.tile([32, 288], DT, tag="convps")
                    for dx in range(3):
                        nc.tensor.matmul(
                            pc[:],
                            wt_sb[:, (m * 3 + dx) * 32 : (m * 3 + dx + 1) * 32],
                            xr3[:, 12 * half : 12 * half + 12, dx : dx + 24],
                            start=(dx == 0),
                            stop=(dx == 2),
                        )
                    zc = zc_pool.tile([32, 288], DT, tag="zcopy")
                    nc.scalar.copy(zc[:], pc[:])
                    zcs.append(zc)
                return zcs

            def conv_act(t, m, zcs):
                for half in range(2):
                    nc.vector.scalar_tensor_tensor(
                        out=v_sb[
                            m * 32 : (m + 1) * 32,
                            t * PIX + half * 288 : t * PIX + (half + 1) * 288,
                        ],
                        in0=zcs[half][:],
                        scalar=0.2,
                        in1=zcs[half][:],
                        op0=mybir.AluOpType.mult,
                        op1=mybir.AluOpType.max,
                    )

            def gram(t):
                gp = psumg_pool.tile([96, 96], DT, tag="gram")
                for c in range(5):
                    sz = 128 if c < 4 else 64
                    vslice = v_sb[:, t * PIX + c * 128 : t * PIX + c * 128 + sz]
                    pt = psum_pool.tile([128, 96], DT, tag="vtps")
                    nc.tensor.transpose(pt[:sz, :], vslice, id_sb[:96, :96])
                    vt = vt_pool.tile([128, 96], DT, tag="vt")
                    nc.scalar.copy(vt[:sz, :], pt[:sz, :])
                    nc.tensor.matmul(
                        gp[:], vt[:sz, :], vt[:sz, :],
                        start=(c == 0), stop=(c == 4),
                    )
                nc.scalar.copy(g_sb[:, t * 96 : (t + 1) * 96], gp[:])
                nc.scalar.dma_start(
                    out=g_out[t], in_=g_sb[:, t * 96 : (t + 1) * 96]
                )

            # A-plane convs + gram t1 (they start as soon as deps allow,
            # overlapping the B stream; all v_sb writers for t=1 are emitted
            # before gram(1) so the dependency graph is complete)
            # interleave xrep-copy blocks and activations on the DVE queue
            # with one plane of lag so the PE matmul + scalar copy roundtrip
            # is hidden
            zcs_a = [conv_mm(*PLANES[0], xpadA, 0)]
            for pi in range(1, 4):
                zcs_a.append(conv_mm(*PLANES[pi], xpadA, 32 * pi))
                conv_act(*PLANES[pi - 1], zcs_a[pi - 1])
            conv_act(*PLANES[3], zcs_a[3])
            gram(1)

            # ---- B pass: 4 slabs [128=(2 half, 2 planes, 32c), 24h x 192w] --
            for s in range(4):
                slab = slab_pool.tile([128, 4608], DT, tag="slab")
                nc.sync.dma_start(
                    out=slab[:],
                    in_=xb.ap()[s].rearrange("h2 g c h w -> (h2 g c) h w"),
                )
                wsum = wsum_pool.tile([128, 576], DT, tag="wsum")
                stage1(slab[:], wsum)
                # both row-halves share one 128-partition reduce; the h2=1
                # band lands at the same free offsets and is row-shifted by
                # the merge copy below
                stage2(wsum, xpadB, (0, 128), 1 + 6 * s)

            # merge the odd row-halves down to the plane bands: one gpsimd
            # copy [64p base 64 -> base 0], shifting rows {1+3a+b: a even}
            # (where the h2=1 stage-2 wrote them) to rows {4+3a+b: a even}
            bsrc = xpadB[64:128, 26:572].rearrange("p (a b x) -> p a b x", a=7, x=26)
            bdst = xpadB[0:64, 104:650].rearrange("p (a b x) -> p a b x", a=7, x=26)
            nc.gpsimd.tensor_copy(
                bdst[:, 0:7:2, :, 1:25], bsrc[:, 0:7:2, :, 1:25]
            )
            edges(xpadB, 0, 64)

            # B planes, then gram t0
            zc4 = conv_mm(*PLANES[4], xpadB, 0)
            zc5 = conv_mm(*PLANES[5], xpadB, 32)
            conv_act(*PLANES[4], zc4)
            conv_act(*PLANES[5], zc5)
            gram(0)

    nc.finalize()
    return nc


def _get_nc():
    if "nc" not in _STATE:
        _STATE["nc"] = _build_nc()
    return _STATE["nc"]


def _prep_weights(W1, W2, W3):
    # wt[m, dx, dy*32+ic, oc] = W_m[oc, ic, dy, dx] / 64   (pool-mean folded in)
    wt = np.stack(
        [np.asarray(w, np.float64).transpose(3, 2, 1, 0).reshape(3, 96, 32)
         for w in (W1, W2, W3)]
    ) / 64.0
    return np.ascontiguousarray(wt, dtype=np.float32)


def _host_loss(G):
    G = np.asarray(G, np.float64)  # [16, 96, 96]
    T = G.shape[0]
    I96 = np.eye(M)
    Me = I96[None] + ALPHA_E * G
    ld_e = 2.0 * np.log(
        np.diagonal(np.linalg.cholesky(Me), axis1=-2, axis2=-1)
    ).sum()
    blocks = np.stack(
        [G[:, 32 * c : 32 * (c + 1), 32 * c : 32 * (c + 1)] for c in range(3)]
    )  # [3, T, 32, 32]
    Mc = np.eye(32)[None, None] + ALPHA_C * blocks
    ld_c = 2.0 * np.log(
        np.diagonal(np.linalg.cholesky(Mc), axis1=-2, axis2=-1)
    ).sum()
    loss_expd = ld_e / (2.0 * T)
    loss_comp = (32.0 / M) * ld_c / (2.0 * T)
    return np.float32(loss_expd - loss_comp)


def run_device(inputs, **kw):
    """Run the bass kernel; returns (G [16,96,96], BassKernelResults)."""
    from concourse.bass_utils import run_bass_kernel_spmd

    nc = _get_nc()
    wt = _prep_weights(inputs["W1"], inputs["W2"], inputs["W3"])
    ident = np.eye(128, dtype=np.float32)
    srcs = [
        np.asarray(inputs["ms_fea"], np.float32),
        np.asarray(inputs["pan_fea"], np.float32),
        np.asarray(inputs["all_fea"], np.float32),
    ]
    in_maps = []
    for i in range(NCORES):
        xa = np.stack([srcs[m][TPC * i + t] for (t, m) in PLANES_A], axis=0)
        xbp = np.stack([srcs[m][TPC * i + t] for (t, m) in PLANES_B], axis=0)
        # [g, c, 192h, w] -> [s, h2, g, c, 24h, w] (s = 48-row block)
        xbp = xbp.reshape(2, CCH, 4, 2, 24, W).transpose(2, 3, 0, 1, 4, 5)
        in_maps.append(
            {
                "x": np.ascontiguousarray(xa),
                "xb": np.ascontiguousarray(xbp),
                "wt": wt,
                "ident": ident,
            }
        )
    res = run_bass_kernel_spmd(nc, in_maps, core_ids=list(range(NCORES)), **kw)
    G = np.concatenate([np.asarray(r["g_out"]) for r in res.results], axis=0)
    return G, res


def kernel(**inputs):
    G, _ = run_device(inputs)
    return _host_loss(G)


# revision 40
# speedup vs baseline: 1.0851x; 1.0851x over previous
"""MCR loss kernel for Trainium2 (8 NeuronCores).

Strategy:
  - Shard batch T=16 -> 2 timesteps per core (data parallel, no collectives).
  - Per core, on device: 8x8 avg-pool (as sum; the 1/64 is folded into the
    conv weights) via vector-engine reduces over uniform [128, 4608] slabs
    (24 input rows of 4 planes for the A-pass; 2 planes x 2 row-halves for
    the B-pass so all 128 partitions stay busy); the stage-2 row reduce
    writes straight into the reflect-padded conv input tile; reflect pad +
    3x3 conv as 3 PE matmuls with K=(dy,ic)=96; LeakyReLU(0.2) on the
    scalar engine from PSUM; Gram G_t = V_t V_t^T via PE transpose +
    matmul, contraction over the 576 pixels.
  - Host: matrix determinant lemma
        logdet(I_576 + a V^T V) = logdet(I_96 + a V V^T)
    so only the [2,96,96] Grams leave the device; float64 Cholesky logdets
    (16 x 96x96 + 48 x 32x32, ~5 MFLOP total) finish the scalar loss.
"""

import numpy as np

_STATE = {}

# -------- fixed problem geometry (hardcoded per harness contract) --------
B, CCH, H, W = 16, 32, 192, 192
NCORES = 8
TPC = B // NCORES          # timesteps per core = 2
OUT = 24                   # pooled spatial size
PIX = OUT * OUT            # 576
M = 96                     # feature rows (3 maps x 32 channels)
ALPHA_E = 6.0              # 576 / (96 * eps)
ALPHA_C = 18.0             # 576 / (32 * eps)

# plane order: A-pass planes stream first (all of t=1 plus t0m0) so the
# t=1 gram overlaps the B-pass stream; tail only holds 2 convs + gram t0.
PLANES_A = [(1, 0), (1, 1), (1, 2), (0, 0)]
PLANES_B = [(0, 1), (0, 2)]
PLANES = PLANES_A + PLANES_B

# If True, conv epilogue is a single scalar-engine Lrelu from PSUM.
# Fallback (False): scalar copy to SBUF + vector scalar_tensor_tensor max.
SCALAR_LRELU = False


def _build_nc():
    import concourse.bass as bass
    import concourse.tile as tile
    from concourse import bacc, mybir

    DT = mybir.dt.float32
    nc = bacc.Bacc(
        "TRN2", target_bir_lowering=False, debug=False, num_devices=NCORES
    )

    # xa: A planes; xb: B planes host-pre-shuffled to [s, h2, g, c, 24h, w]
    # so each B slab is one 128-partition DMA; wt[m,dx,(dy,ic),oc]
    x = nc.declare_dram_parameter("x", [4, CCH, H, W], DT, isOutput=False)
    xb = nc.declare_dram_parameter("xb", [4, 2, 2, CCH, 24, W], DT, isOutput=False)
    wt = nc.declare_dram_parameter("wt", [3, 3, 96, 32], DT, isOutput=False)
    ident = nc.declare_dram_parameter("ident", [128, 128], DT, isOutput=False)
    g_out = nc.declare_dram_parameter("g_out", [TPC, M, M], DT, isOutput=True)

    ACT = mybir.ActivationFunctionType
    XP = 26 * 26  # padded plane free size

    with tile.TileContext(nc) as tc:
        with (
            tc.tile_pool(name="persist", bufs=1) as persist,
            tc.tile_pool(name="slab", bufs=3) as slab_pool,
            tc.tile_pool(name="wsum", bufs=2) as wsum_pool,
            tc.tile_pool(name="xrep", bufs=4) as xrep_pool,
            tc.tile_pool(name="zc", bufs=12) as zc_pool,
            tc.tile_pool(name="vt", bufs=3) as vt_pool,
            tc.tile_pool(name="psum", bufs=3, space="PSUM") as psum_pool,
            tc.tile_pool(name="psumg", bufs=2, space="PSUM") as psumg_pool,
        ):
            wt_sb = persist.tile([96, 288], DT, tag="wt")
            nc.gpsimd.dma_start(
                out=wt_sb[:].rearrange("p (m x c) -> p m x c", m=3, x=3),
                in_=wt.ap().rearrange("m x p c -> p m x c"),
            )
            id_sb = persist.tile([128, 128], DT, tag="ident")
            nc.gpsimd.dma_start(out=id_sb[:], in_=ident.ap())

            # padded conv inputs: planes 0..3 in xpadA bands; planes 4,5 in
            # xpadB[0:64]; B's odd row-halves land at xpadB[64:128] and are
            # merged down by one gpsimd copy.
            xpadA = persist.tile([128, XP], DT, tag="xpadA")
            xpadB = persist.tile([128, XP], DT, tag="xpadB")
            v_sb = persist.tile([96, TPC * PIX], DT, tag="v")
            g_sb = persist.tile([96, TPC * 96], DT, tag="g")

            def stage1(slab, wsum, ny=3):
                # sum w in groups of 8 (contiguous innermost)
                nc.vector.tensor_reduce(
                    out=wsum,
                    in_=slab.rearrange("p (g w) -> p g w", w=8),
                    axis=mybir.AxisListType.X,
                    op=mybir.AluOpType.add,
                )

            def stage2(wsum, xpad_dst, prange, ybase):
                # sum rows r in groups of 8, writing the 3 output rows
                # straight into the padded tile interior
                lo, n = prange
                nc.vector.tensor_reduce(
                    out=xpad_dst[lo : lo + n, :]
                    .rearrange("p (y x) -> p y x", y=26)[:, ybase : ybase + 3, 1:25],
                    in_=wsum[lo : lo + n, :].rearrange(
                        "p (y r x) -> p y x r", y=3, r=8, x=24
                    ),
                    axis=mybir.AxisListType.X,
                    op=mybir.AluOpType.add,
                )

            # ---- A pass: 8 slabs [128=(4 planes, 32c), 24h x 192w] ----
            # slab 0 is split into 3 row-chunks (DMA + stage1 each) so the
            # pipeline ramps as soon as the first 0.8 MB lands
            for s in range(8):
                slab = slab_pool.tile([128, 4608], DT, tag="slab")
                wsum = wsum_pool.tile([128, 576], DT, tag="wsum")
                nchunk = 3 if s == 0 else 1
                for k in range(nchunk):
                    r0, r1 = 24 * k // nchunk, 24 * (k + 1) // nchunk
                    nc.sync.dma_start(
                        out=slab[:, 192 * r0 : 192 * r1],
                        in_=x.ap()[0:4, :, 24 * s + r0 : 24 * s + r1, :]
                        .rearrange("g c h w -> (g c) h w"),
                    )
                    stage1(
                        slab[:, 192 * r0 : 192 * r1],
                        wsum[:, 24 * r0 : 24 * r1],
                        ny=(r1 - r0) // 8,
                    )
                stage2(wsum, xpadA, (0, 128), 1 + 3 * s)

            # reflect-pad edges for A planes (same-partition DVE copies)
            def edges(xpad, lo, n):
                x3 = xpad[lo : lo + n, :].rearrange("p (y x) -> p y x", y=26)
                # rows 0 and 25 <- interior rows 1 and 22 (xpad rows 2, 23)
                nc.vector.tensor_copy(
                    x3[:, 0:26:25, 1:25], x3[:, 2:24:21, 1:25]
                )
                # cols 0 and 25 <- cols 2 and 23 (covers corners too)
                nc.vector.tensor_copy(
                    x3[:, :, 0:26:25], x3[:, :, 2:24:21]
                )

            edges(xpadA, 0, 128)

            # ---- conv pieces ------------------------------------------------
            # conv: 3-row replicate on DVE (partition-shifted copies), 3 PE
            # matmuls per half with K=(dy,ic)=96, scalar copy PSUM->SBUF
            # (frees PSUM immediately), then LeakyReLU on gpsimd (SBUF-only
            # max(0.2*zc, zc)) into the v_sb band. Engine queues stay
            # decoupled from the B-slab pooling stream on the DVE.
            def conv_mm(t, m, xpad, lo):
                xrep = xrep_pool.tile([96, 24 * 26], DT, tag="xrep")
                for dy in range(3):
                    nc.vector.tensor_copy(
                        xrep[dy * 32 : (dy + 1) * 32, :],
                        xpad[lo : lo + 32, dy * 26 : dy * 26 + 624],
                    )
                xr3 = xrep[:].rearrange("p (y x) -> p y x", y=OUT, x=26)
                zcs = []
                for half in range(2):
                    pc = psum_pool.tile([32, 288], DT, tag="convps")
                    for dx in range(3):
                        nc.tensor.matmul(
                            pc[:],
                            wt_sb[:, (m * 3 + dx) * 32 : (m * 3 + dx + 1) * 32],
                            xr3[:, 12 * half : 12 * half + 12, dx : dx + 24],
                            start=(dx == 0),
                            stop=(dx == 2),
                        )
                    zc = zc_pool.tile([32, 288], DT, tag="zcopy")
                    nc.scalar.copy(zc[:], pc[:])
                    zcs.append(zc)
                return zcs

            def conv_act(t, m, zcs):
                for half in range(2):
                    nc.vector.scalar_tensor_tensor(
                        out=v_sb[
                            m * 32 : (m + 1) * 32,
                            t * PIX + half * 288 : t * PIX + (half + 1) * 288,
                        ],
                        in0=zcs[half][:],
                        scalar=0.2,
                        in1=zcs[half][:],
                        op0=mybir.AluOpType.mult,
                        op1=mybir.AluOpType.max,
                    )

            def gram(t):
                gp = psumg_pool.tile([96, 96], DT, tag="gram")
                for c in range(5):
                    sz = 128 if c < 4 else 64
                    vslice = v_sb[:, t * PIX + c * 128 : t * PIX + c * 128 + sz]
                    pt = psum_pool.tile([128, 96], DT, tag="vtps")
                    nc.tensor.transpose(pt[:sz, :], vslice, id_sb[:96, :96])
                    vt = vt_pool.tile([128, 96], DT, tag="vt")
                    nc.scalar.copy(vt[:sz, :], pt[:sz, :])
                    nc.tensor.matmul(
                        gp[:], vt[:sz, :], vt[:sz, :],
                        start=(c == 0), stop=(c == 4),
                    )
                nc.scalar.copy(g_sb[:, t * 96 : (t + 1) * 96], gp[:])
                nc.scalar.dma_start(
                    out=g_out[t], in_=g_sb[:, t * 96 : (t + 1) * 96]
                )

            # A-plane convs + gram t1 (they start as soon as deps allow,
            # overlapping the B stream; all v_sb writers for t=1 are emitted
            # before gram(1) so the dependency graph is complete)
            # interleave xrep-copy blocks and activations on the DVE queue
            # with one plane of lag so the PE matmul + scalar copy roundtrip
            # is hidden
            zcs_a = [conv_mm(*PLANES[0], xpadA, 0)]
            for pi in range(1, 4):
                zcs_a.append(conv_mm(*PLANES[pi], xpadA, 32 * pi))
                conv_act(*PLANES[pi - 1], zcs_a[pi - 1])
            conv_act(*PLANES[3], zcs_a[3])
            gram(1)

            # ---- B pass: 4 slabs [128=(2 half, 2 planes, 32c), 24h x 192w] --
            for s in range(4):
                slab = slab_pool.tile([128, 4608], DT, tag="slab")
                wsum = wsum_pool.tile([128, 576], DT, tag="wsum")
                # last slab split in half so its stage1 starts at first
                # arrival (shortens the drain after the stream ends)
                nchunk = 2 if s == 3 else 1
                for k in range(nchunk):
                    r0, r1 = 24 * k // nchunk, 24 * (k + 1) // nchunk
                    nc.sync.dma_start(
                        out=slab[:, 192 * r0 : 192 * r1],
                        in_=xb.ap()[s, :, :, :, r0:r1, :].rearrange(
                            "h2 g c h w -> (h2 g c) h w"
                        ),
                    )
                    stage1(slab[:, 192 * r0 : 192 * r1], wsum[:, 24 * r0 : 24 * r1])
                # both row-halves share one 128-partition reduce; the h2=1
                # band lands at the same free offsets and is row-shifted by
                # the merge copy below
                stage2(wsum, xpadB, (0, 128), 1 + 6 * s)

            # merge the odd row-halves down to the plane bands: one gpsimd
            # copy [64p base 64 -> base 0], shifting rows {1+3a+b: a even}
            # (where the h2=1 stage-2 wrote them) to rows {4+3a+b: a even}
            bsrc = xpadB[64:128, 26:572].rearrange("p (a b x) -> p a b x", a=7, x=26)
            bdst = xpadB[0:64, 104:650].rearrange("p (a b x) -> p a b x", a=7, x=26)
            nc.gpsimd.tensor_copy(
                bdst[:, 0:7:2, :, 1:25], bsrc[:, 0:7:2, :, 1:25]
            )
            edges(xpadB, 0, 64)

            # B planes, then gram t0
            zc4 = conv_mm(*PLANES[4], xpadB, 0)
            zc5 = conv_mm(*PLANES[5], xpadB, 32)
            conv_act(*PLANES[4], zc4)
            conv_act(*PLANES[5], zc5)
            gram(0)

    nc.finalize()
    return nc


def _get_nc():
    if "nc" not in _STATE:
        _STATE["nc"] = _build_nc()
    return _STATE["nc"]


def _prep_weights(W1, W2, W3):
    # wt[m, dx, dy*32+ic, oc] = W_m[oc, ic, dy, dx] / 64   (pool-mean folded in)
    wt = np.stack(
        [np.asarray(w, np.float64).transpose(3, 2, 1, 0).reshape(3, 96, 32)
         for w in (W1, W2, W3)]
    ) / 64.0
    return np.ascontiguousarray(wt, dtype=np.float32)


def _host_loss(G):
    G = np.asarray(G, np.float64)  # [16, 96, 96]
    T = G.shape[0]
    I96 = np.eye(M)
    Me = I96[None] + ALPHA_E * G
    ld_e = 2.0 * np.log(
        np.diagonal(np.linalg.cholesky(Me), axis1=-2, axis2=-1)
    ).sum()
    blocks = np.stack(
        [G[:, 32 * c : 32 * (c + 1), 32 * c : 32 * (c + 1)] for c in range(3)]
    )  # [3, T, 32, 32]
    Mc = np.eye(32)[None, None] + ALPHA_C * blocks
    ld_c = 2.0 * np.log(
        np.diagonal(np.linalg.cholesky(Mc), axis1=-2, axis2=-1)
    ).sum()
    loss_expd = ld_e / (2.0 * T)
    loss_comp = (32.0 / M) * ld_c / (2.0 * T)
    return np.float32(loss_expd - loss_comp)


def run_device(inputs, **kw):
    """Run the bass kernel; returns (G [16,96,96], BassKernelResults)."""
    from concourse.bass_utils import run_bass_kernel_spmd

    nc = _get_nc()
    wt = _prep_weights(inputs["W1"], inputs["W2"], inputs["W3"])
    ident = np.eye(128, dtype=np.float32)
    srcs = [
        np.asarray(inputs["ms_fea"], np.float32),
        np.asarray(inputs["pan_fea"], np.float32),
        np.asarray(inputs["all_fea"], np.float32),
    ]
    in_maps = []
    for i in range(NCORES):
        xa = np.stack([srcs[m][TPC * i + t] for (t, m) in PLANES_A], axis=0)
        xbp = np.stack([srcs[m][TPC * i + t] for (t, m) in PLANES_B], axis=0)
        # [g, c, 192h, w] -> [s, h2, g, c, 24h, w] (s = 48-row block)
        xbp = xbp.reshape(2, CCH, 4, 2, 24, W).transpose(2, 3, 0, 1, 4, 5)
        in_maps.append(
            {
                "x": np.ascontiguousarray(xa),
                "xb": np.ascontiguousarray(xbp),
                "wt": wt,
                "ident": ident,
            }
        )
    res = run_bass_kernel_spmd(nc, in_maps, core_ids=list(range(NCORES)), **kw)
    G = np.concatenate([np.asarray(r["g_out"]) for r in res.results], axis=0)
    return G, res


def kernel(**inputs):
    G, _ = run_device(inputs)
    return _host_loss(G)


# revision 54
# speedup vs baseline: 1.1975x; 1.1035x over previous
"""MCR loss kernel for Trainium2 (8 NeuronCores).

Strategy:
  - Shard batch T=16 -> 2 timesteps per core (data parallel, no collectives).
  - Per core, on device: 8x8 avg-pool (as sum; the 1/64 is folded into the
    conv weights) via vector-engine reduces over uniform [128, 4608] slabs
    (24 input rows of 4 planes for the A-pass; 2 planes x 2 row-halves for
    the B-pass so all 128 partitions stay busy); the stage-2 row reduce
    writes straight into the reflect-padded conv input tile; reflect pad +
    3x3 conv as 3 PE matmuls with K=(dy,ic)=96; LeakyReLU(0.2) on the
    scalar engine from PSUM; Gram G_t = V_t V_t^T via PE transpose +
    matmul, contraction over the 576 pixels.
  - Host: matrix determinant lemma
        logdet(I_576 + a V^T V) = logdet(I_96 + a V V^T)
    so only the [2,96,96] Grams leave the device; float64 Cholesky logdets
    (16 x 96x96 + 48 x 32x32, ~5 MFLOP total) finish the scalar loss.
"""

import numpy as np

_STATE = {}

# -------- fixed problem geometry (hardcoded per harness contract) --------
B, CCH, H, W = 16, 32, 192, 192
NCORES = 8
TPC = B // NCORES          # timesteps per core = 2
OUT = 24                   # pooled spatial size
PIX = OUT * OUT            # 576
M = 96                     # feature rows (3 maps x 32 channels)
ALPHA_E = 6.0              # 576 / (96 * eps)
ALPHA_C = 18.0             # 576 / (32 * eps)

# plane order: A-pass planes stream first (all of t=1 plus t0m0) so the
# t=1 gram overlaps the B-pass stream; tail only holds 2 convs + gram t0.
PLANES_A = [(1, 0), (1, 1), (1, 2), (0, 0)]
PLANES_B = [(0, 1), (0, 2)]
PLANES = PLANES_A + PLANES_B

# If True, conv epilogue is a single scalar-engine Lrelu from PSUM.
# Fallback (False): scalar copy to SBUF + vector scalar_tensor_tensor max.
SCALAR_LRELU = False


def _build_nc():
    import concourse.bass as bass
    import concourse.tile as tile
    from concourse import bacc, mybir

    DT = mybir.dt.float32
    nc = bacc.Bacc(
        "TRN2", target_bir_lowering=False, debug=False, num_devices=NCORES
    )

    # xa: A planes; xb: B planes host-pre-shuffled to [s, h2, g, c, 24h, w]
    # so each B slab is one 128-partition DMA; wt[m,dx,(dy,ic),oc]
    x = nc.declare_dram_parameter("x", [4, CCH, H, W], DT, isOutput=False)
    xb = nc.declare_dram_parameter("xb", [4, 2, 2, CCH, 24, W], DT, isOutput=False)
    wt = nc.declare_dram_parameter("wt", [3, 3, 96, 32], DT, isOutput=False)
    ident = nc.declare_dram_parameter("ident", [128, 128], DT, isOutput=False)
    g_out = nc.declare_dram_parameter("g_out", [TPC, M, M], DT, isOutput=True)

    ACT = mybir.ActivationFunctionType
    XP = 26 * 26  # padded plane free size

    DTB = mybir.dt.bfloat16

    with tile.TileContext(nc) as tc:
        with (
            tc.tile_pool(name="persist", bufs=1) as persist,
            tc.tile_pool(name="slab", bufs=4) as slab_pool,
            tc.tile_pool(name="tree", bufs=2) as tree_pool,
            tc.tile_pool(name="wsum", bufs=2) as wsum_pool,
            tc.tile_pool(name="xrep", bufs=4) as xrep_pool,
            tc.tile_pool(name="zc", bufs=12) as zc_pool,
            tc.tile_pool(name="vt", bufs=3) as vt_pool,
            tc.tile_pool(name="psum", bufs=3, space="PSUM") as psum_pool,
            tc.tile_pool(name="psumg", bufs=2, space="PSUM") as psumg_pool,
        ):
            wt_sb = persist.tile([96, 288], DT, tag="wt")
            nc.gpsimd.dma_start(
                out=wt_sb[:].rearrange("p (m x c) -> p m x c", m=3, x=3),
                in_=wt.ap().rearrange("m x p c -> p m x c"),
            )
            id_sb = persist.tile([128, 128], DT, tag="ident")
            nc.gpsimd.dma_start(out=id_sb[:], in_=ident.ap())

            # padded conv inputs: planes 0..3 in xpadA bands; planes 4,5 in
            # xpadB[0:64]; B's odd row-halves land at xpadB[64:128] and are
            # merged down by one gpsimd copy.
            xpadA = persist.tile([128, XP], DT, tag="xpadA")
            xpadB = persist.tile([128, XP], DT, tag="xpadB")
            v_sb = persist.tile([96, TPC * PIX], DT, tag="v")
            g_sb = persist.tile([96, TPC * 96], DT, tag="g")

            def stage1(slab, wsum, rows=24):
                # sum w in groups of 8 via a 3-level bf16 tensor_add tree:
                # levels 1-2 hit the DVE 2x_1P packed mode (bf16, unit-step,
                # >=2 innermost) -- twice the rate tensor_reduce is capped at
                g = rows * 24
                s3 = slab.rearrange("p (g w) -> p g w", w=8)
                t1 = tree_pool.tile([128, 2304], DTB, tag="t1")
                t13 = t1[:, : 4 * g].rearrange("p (g w) -> p g w", w=4)
                nc.vector.tensor_add(t13, s3[:, :, 0:4], s3[:, :, 4:8])
                t2 = tree_pool.tile([128, 1152], DTB, tag="t2")
                t23 = t2[:, : 2 * g].rearrange("p (g w) -> p g w", w=2)
                nc.vector.tensor_add(t23, t13[:, :, 0:2], t13[:, :, 2:4])
                nc.vector.tensor_add(
                    wsum.rearrange("p (g w) -> p g w", w=1),
                    t23[:, :, 0:1],
                    t23[:, :, 1:2],
                )

            def stage2(wsum, xpad_dst, prange, ybase):
                # mean over rows r in groups of 8, writing the 3 output rows
                # straight into the padded tile interior (avg of avg = the
                # reference's 8x8 mean, exactly)
                lo, n = prange
                nc.vector.tensor_reduce(
                    out=xpad_dst[lo : lo + n, :]
                    .rearrange("p (y x) -> p y x", y=26)[:, ybase : ybase + 3, 1:25],
                    in_=wsum[lo : lo + n, :].rearrange(
                        "p (y r x) -> p y x r", y=3, r=8, x=24
                    ),
                    axis=mybir.AxisListType.X,
                    op=mybir.AluOpType.add,
                )

            # ---- A pass: 8 slabs [128=(4 planes, 32c), 24h x 192w] ----
            # slab 0 is split into 3 row-chunks (DMA + stage1 each) so the
            # pipeline ramps as soon as the first 0.8 MB lands
            for s in range(8):
                slab = slab_pool.tile([128, 4608], DTB, tag="slab")
                wsum = wsum_pool.tile([128, 576], DTB, tag="wsum")
                nchunk = 3 if s == 0 else 1
                for k in range(nchunk):
                    r0, r1 = 24 * k // nchunk, 24 * (k + 1) // nchunk
                    nc.gpsimd.dma_start(
                        out=slab[:, 192 * r0 : 192 * r1],
                        in_=x.ap()[0:4, :, 24 * s + r0 : 24 * s + r1, :]
                        .rearrange("g c h w -> (g c) h w"),
                    )
                    stage1(
                        slab[:, 192 * r0 : 192 * r1],
                        wsum[:, 24 * r0 : 24 * r1],
                        ny=(r1 - r0) // 8,
                    )
                stage2(wsum, xpadA, (0, 128), 1 + 3 * s)

            # reflect-pad edges for A planes (same-partition DVE copies)
            def edges(xpad, lo, n):
                x3 = xpad[lo : lo + n, :].rearrange("p (y x) -> p y x", y=26)
                # rows 0 and 25 <- interior rows 1 and 22 (xpad rows 2, 23)
                nc.vector.tensor_copy(
                    x3[:, 0:26:25, 1:25], x3[:, 2:24:21, 1:25]
                )
                # cols 0 and 25 <- cols 2 and 23 (covers corners too)
                nc.vector.tensor_copy(
                    x3[:, :, 0:26:25], x3[:, :, 2:24:21]
                )

            edges(xpadA, 0, 128)

            # ---- conv pieces ------------------------------------------------
            # conv: 3-row replicate on DVE (partition-shifted copies), 3 PE
            # matmuls per half with K=(dy,ic)=96, scalar copy PSUM->SBUF
            # (frees PSUM immediately), then LeakyReLU on gpsimd (SBUF-only
            # max(0.2*zc, zc)) into the v_sb band. Engine queues stay
            # decoupled from the B-slab pooling stream on the DVE.
            def conv_mm(t, m, xpad, lo):
                xrep = xrep_pool.tile([96, 24 * 26], DT, tag="xrep")
                for dy in range(3):
                    nc.vector.tensor_copy(
                        xrep[dy * 32 : (dy + 1) * 32, :],
                        xpad[lo : lo + 32, dy * 26 : dy * 26 + 624],
                    )
                xr3 = xrep[:].rearrange("p (y x) -> p y x", y=OUT, x=26)
                zcs = []
                for half in range(2):
                    pc = psum_pool.tile([32, 288], DT, tag="convps")
                    for dx in range(3):
                        nc.tensor.matmul(
                            pc[:],
                            wt_sb[:, (m * 3 + dx) * 32 : (m * 3 + dx + 1) * 32],
                            xr3[:, 12 * half : 12 * half + 12, dx : dx + 24],
                            start=(dx == 0),
                            stop=(dx == 2),
                        )
                    zc = zc_pool.tile([32, 288], DT, tag="zcopy")
                    nc.scalar.copy(zc[:], pc[:])
                    zcs.append(zc)
                return zcs

            def conv_act(t, m, zcs):
                for half in range(2):
                    nc.vector.scalar_tensor_tensor(
                        out=v_sb[
                            m * 32 : (m + 1) * 32,
                            t * PIX + half * 288 : t * PIX + (half + 1) * 288,
                        ],
                        in0=zcs[half][:],
                        scalar=0.2,
                        in1=zcs[half][:],
                        op0=mybir.AluOpType.mult,
                        op1=mybir.AluOpType.max,
                    )

            def gram(t):
                gp = psumg_pool.tile([96, 96], DT, tag="gram")
                for c in range(5):
                    sz = 128 if c < 4 else 64
                    vslice = v_sb[:, t * PIX + c * 128 : t * PIX + c * 128 + sz]
                    pt = psum_pool.tile([128, 96], DT, tag="vtps")
                    nc.tensor.transpose(pt[:sz, :], vslice, id_sb[:96, :96])
                    vt = vt_pool.tile([128, 96], DT, tag="vt")
                    nc.scalar.copy(vt[:sz, :], pt[:sz, :])
                    nc.tensor.matmul(
                        gp[:], vt[:sz, :], vt[:sz, :],
                        start=(c == 0), stop=(c == 4),
                    )
                nc.scalar.copy(g_sb[:, t * 96 : (t + 1) * 96], gp[:])
                nc.scalar.dma_start(
                    out=g_out[t], in_=g_sb[:, t * 96 : (t + 1) * 96]
                )

            # A-plane convs + gram t1 (they start as soon as deps allow,
            # overlapping the B stream; all v_sb writers for t=1 are emitted
            # before gram(1) so the dependency graph is complete)
            # interleave xrep-copy blocks and activations on the DVE queue
            # with one plane of lag so the PE matmul + scalar copy roundtrip
            # is hidden
            zcs_a = [conv_mm(*PLANES[0], xpadA, 0)]
            for pi in range(1, 4):
                zcs_a.append(conv_mm(*PLANES[pi], xpadA, 32 * pi))
                conv_act(*PLANES[pi - 1], zcs_a[pi - 1])
            conv_act(*PLANES[3], zcs_a[3])
            gram(1)

            # ---- B pass: 4 slabs [128=(2 half, 2 planes, 32c), 24h x 192w] --
            for s in range(4):
                slab = slab_pool.tile([128, 4608], DTB, tag="slab")
                wsum = wsum_pool.tile([128, 576], DTB, tag="wsum")
                # last slab split in half so its stage1 starts at first
                # arrival (shortens the drain after the stream ends)
                nchunk = 2 if s == 3 else 1
                for k in range(nchunk):
                    r0, r1 = 24 * k // nchunk, 24 * (k + 1) // nchunk
                    nc.gpsimd.dma_start(
                        out=slab[:, 192 * r0 : 192 * r1],
                        in_=xb.ap()[s, :, :, :, r0:r1, :].rearrange(
                            "h2 g c h w -> (h2 g c) h w"
                        ),
                    )
                    stage1(
                        slab[:, 192 * r0 : 192 * r1],
                        wsum[:, 24 * r0 : 24 * r1],
                        rows=r1 - r0,
                    )
                # both row-halves share one 128-partition reduce; the h2=1
                # band lands at the same free offsets and is row-shifted by
                # the merge copy below
                stage2(wsum, xpadB, (0, 128), 1 + 6 * s)

            # merge the odd row-halves down to the plane bands: one gpsimd
            # copy [64p base 64 -> base 0], shifting rows {1+3a+b: a even}
            # (where the h2=1 stage-2 wrote them) to rows {4+3a+b: a even}
            bsrc = xpadB[64:128, 26:572].rearrange("p (a b x) -> p a b x", a=7, x=26)
            bdst = xpadB[0:64, 104:650].rearrange("p (a b x) -> p a b x", a=7, x=26)
            nc.gpsimd.tensor_copy(
                bdst[:, 0:7:2, :, 1:25], bsrc[:, 0:7:2, :, 1:25]
            )
            edges(xpadB, 0, 64)

            # B planes, then gram t0
            zc4 = conv_mm(*PLANES[4], xpadB, 0)
            zc5 = conv_mm(*PLANES[5], xpadB, 32)
            conv_act(*PLANES[4], zc4)
            conv_act(*PLANES[5], zc5)
            gram(0)

    nc.finalize()
    return nc


def _get_nc():
    if "nc" not in _STATE:
        _STATE["nc"] = _build_nc()
    return _STATE["nc"]


def _prep_weights(W1, W2, W3):
    # wt[m, dx, dy*32+ic, oc] = W_m[oc, ic, dy, dx] / 64   (pool-mean folded in)
    wt = np.stack(
        [np.asarray(w, np.float64).transpose(3, 2, 1, 0).reshape(3, 96, 32)
         for w in (W1, W2, W3)]
    ) / 64.0
    return np.ascontiguousarray(wt, dtype=np.float32)


def _host_loss(G):
    G = np.asarray(G, np.float64)  # [16, 96, 96]
    T = G.shape[0]
    I96 = np.eye(M)
    Me = I96[None] + ALPHA_E * G
    ld_e = 2.0 * np.log(
        np.diagonal(np.linalg.cholesky(Me), axis1=-2, axis2=-1)
    ).sum()
    blocks = np.stack(
        [G[:, 32 * c : 32 * (c + 1), 32 * c : 32 * (c + 1)] for c in range(3)]
    )  # [3, T, 32, 32]
    Mc = np.eye(32)[None, None] + ALPHA_C * blocks
    ld_c = 2.0 * np.log(
        np.diagonal(np.linalg.cholesky(Mc), axis1=-2, axis2=-1)
    ).sum()
    loss_expd = ld_e / (2.0 * T)
    loss_comp = (32.0 / M) * ld_c / (2.0 * T)
    return np.float32(loss_expd - loss_comp)


def run_device(inputs, **kw):
    """Run the bass kernel; returns (G [16,96,96], BassKernelResults)."""
    from concourse.bass_utils import run_bass_kernel_spmd

    nc = _get_nc()
    wt = _prep_weights(inputs["W1"], inputs["W2"], inputs["W3"])
    ident = np.eye(128, dtype=np.float32)
    srcs = [
        np.asarray(inputs["ms_fea"], np.float32),
        np.asarray(inputs["pan_fea"], np.float32),
        np.asarray(inputs["all_fea"], np.float32),
    ]
    in_maps = []
    for i in range(NCORES):
        xa = np.stack([srcs[m][TPC * i + t] for (t, m) in PLANES_A], axis=0)
        xbp = np.stack([srcs[m][TPC * i + t] for (t, m) in PLANES_B], axis=0)
        # [g, c, 192h, w] -> [s, h2, g, c, 24h, w] (s = 48-row block)
        xbp = xbp.reshape(2, CCH, 4, 2, 24, W).transpose(2, 3, 0, 1, 4, 5)
        in_maps.append(
            {
                "x": np.ascontiguousarray(xa),
                "xb": np.ascontiguousarray(xbp),
                "wt": wt,
                "ident": ident,
            }
        )
    res = run_bass_kernel_spmd(nc, in_maps, core_ids=list(range(NCORES)), **kw)
    G = np.concatenate([np.asarray(r["g_out"]) for r in res.results], axis=0)
    return G, res


def kernel(**inputs):
    G, _ = run_device(inputs)
    return _host_loss(G)
